# revision 1
# baseline (speedup 1.0000x reference)
"""NonLocalBlock (B=4, C=64, Ci=32, H=W=64) on 8 TRN2 NeuronCores.

Sharding: data-parallel over batch (4 pairs of cores); within each pair
the query dimension n of the NxN score matrix is split in half.
Softmax runs over n (dim=1), so each core computes partial softmax
denominators S[m] over its n-half; tiny pairwise AllReduces ([128 x g]
f32) produce the full denominators. Everything else is local: each
core produces z[:, n_half] and the host concatenates.

Per core (b = core//2, h = core%2):
  theta = theta_w @ supp[:, nh] + theta_b           [32, 2048]  bf16
  phi   = phi_w @ ref + phi_b                       [32, 4096]  bf16
  fT    = phi_tile^T @ theta   (per m-tile of 128)  [128, 2048] PSUM f32
  expT  = exp(fT)  (ACT, accum_out -> row sums)     bf16 SBUF
  S     = AllReduce_pair(row sums)
  wgT   = ref_aug^T @ (w_w@g_w | w_w@g_b)^T         [128, 64] per m-tile
  wgT'  = wgT * (1/S)   (softmax scale + final 1x1 conv folded into g)
  z     = sum_mt wgT'^T @ expT   (PSUM accum)       [64, 2048] f32
  out   = supp[:, nh] + z + w_b
"""

import numpy as np

B, C, CI, H, W = 4, 64, 32, 64, 64
N = H * W            # 4096
NLOC = N // 2        # 2048 n-columns per core
NCORES = 8
MTP = 128            # m-tile partition size
NMT = N // MTP       # 32 m-tiles
GROUP_SIZES = [16, 10, 6]       # penultimate CC lands before B ends
CK = 512             # matmul moving-dim chunk

REPLICA_GROUPS = [[0, 1], [2, 3], [4, 5], [6, 7]]

_cache = {}


def _build():
    import concourse.bacc as bacc
    import concourse.tile as tile
    from concourse import mybir

    f32 = mybir.dt.float32
    bf16 = mybir.dt.bfloat16
    AF = mybir.ActivationFunctionType
    ALU = mybir.AluOpType

    nc = bacc.Bacc(None, target_bir_lowering=False, debug=False)

    supp = nc.dram_tensor("supp", [C, NLOC], f32, kind="ExternalInput")
    supp_b = nc.dram_tensor("supp_b", [C, NLOC], bf16, kind="ExternalInput")
    ref_aug = nc.dram_tensor("ref_aug", [C + 1, N], bf16, kind="ExternalInput")
    theta_wT = nc.dram_tensor("theta_wT", [C, CI], bf16, kind="ExternalInput")
    theta_bc = nc.dram_tensor("theta_bc", [CI, 1], f32, kind="ExternalInput")
    phi_wT = nc.dram_tensor("phi_wT", [C, CI], bf16, kind="ExternalInput")
    phi_bc = nc.dram_tensor("phi_bc", [CI, 1], f32, kind="ExternalInput")
    wg_aug = nc.dram_tensor("wg_aug", [C + 1, C], bf16, kind="ExternalInput")
    w_bc = nc.dram_tensor("w_bc", [C, 1], f32, kind="ExternalInput")
    out = nc.dram_tensor("out", [C, NLOC], f32, kind="ExternalOutput")

    assert sum(GROUP_SIZES) == NMT
    group_of = []
    for g, gs in enumerate(GROUP_SIZES):
        group_of += [g] * gs
    group_start = [sum(GROUP_SIZES[:g]) for g in range(len(GROUP_SIZES))]

    with tile.TileContext(nc) as tc:
        from contextlib import ExitStack

        with ExitStack() as ctx:
            sing = ctx.enter_context(tc.tile_pool(name="sing", bufs=1))
            spool = ctx.enter_context(tc.tile_pool(name="spool", bufs=2))
            epool = ctx.enter_context(tc.tile_pool(name="expT", bufs=NMT))
            dpool = ctx.enter_context(
                tc.tile_pool(name="dram", bufs=len(GROUP_SIZES), space="DRAM")
            )
            outp = ctx.enter_context(tc.tile_pool(name="outp", bufs=3))
            # ftp opened first so it owns PSUM banks 0-3; psA takes 4-7 and
            # is closed mid-B-loop, releasing those banks to the z pool.
            ftp = ctx.enter_context(tc.tile_pool(name="ftp", bufs=2, space="PSUM"))

            # ---------------- loads ----------------
            # big/critical tensors on the sync queue first; small weights
            # issue from gpsimd in parallel
            supp_bf = sing.tile([C, NLOC], bf16, tag="suppbf")
            nc.sync.dma_start(out=supp_bf[0:32, :], in_=supp_b[0:32, :])
            nc.scalar.dma_start(out=supp_bf[32:C, :], in_=supp_b[32:C, :])
            refa = sing.tile([C + 1, N], bf16, tag="refa")
            nc.sync.dma_start(out=refa[0:33, :], in_=ref_aug[0:33, :])
            nc.scalar.dma_start(out=refa[33 : C + 1, :], in_=ref_aug[33 : C + 1, :])
            supp_t = sing.tile([C, NLOC], f32, tag="supp")
            nc.sync.dma_start(out=supp_t, in_=supp[:, :])
            tw = sing.tile([C, CI], bf16, tag="tw")
            nc.gpsimd.dma_start(out=tw, in_=theta_wT[:, :])
            tb = sing.tile([CI, 1], f32, tag="tb")
            nc.gpsimd.dma_start(out=tb, in_=theta_bc[:, :])
            pw = sing.tile([C, CI], bf16, tag="pw")
            nc.gpsimd.dma_start(out=pw, in_=phi_wT[:, :])
            pb = sing.tile([CI, 1], f32, tag="pb")
            nc.gpsimd.dma_start(out=pb, in_=phi_bc[:, :])
            wga = sing.tile([C + 1, C], bf16, tag="wga")
            nc.gpsimd.dma_start(out=wga, in_=wg_aug[:, :])
            wb = sing.tile([C, 1], f32, tag="wb")
            nc.gpsimd.dma_start(out=wb, in_=w_bc[:, :])

            theta_t = sing.tile([CI, NLOC], bf16, tag="theta")
            phi_t = sing.tile([CI, N], bf16, tag="phi")
            wgt_raw = sing.tile([MTP, NMT * C], f32, tag="wgtraw")
            wgt_b16 = sing.tile([MTP, NMT * C], bf16, tag="wgtb16")

            psA_ctx = ExitStack()
            psA = psA_ctx.enter_context(
                tc.tile_pool(name="psA", bufs=2, space="PSUM")
            )

            # -------- phase A: theta/phi projections only --------
            for j in range(NLOC // CK):
                ps = psA.tile([CI, CK], f32, tag="projps")
                nc.tensor.matmul(
                    ps,
                    lhsT=tw[:, :],
                    rhs=supp_bf[:, j * CK : (j + 1) * CK],
                    start=True,
                    stop=True,
                )
                nc.vector.tensor_scalar_add(
                    theta_t[:, j * CK : (j + 1) * CK], ps, tb[:, :]
                )
            def emit_phi(j):
                ps = psA.tile([CI, CK], f32, tag="projps", name=f"phi_ps{j}")
                nc.tensor.matmul(
                    ps,
                    lhsT=pw[:, :],
                    rhs=refa[0:C, j * CK : (j + 1) * CK],
                    start=True,
                    stop=True,
                )
                nc.vector.tensor_scalar_add(
                    phi_t[:, j * CK : (j + 1) * CK], ps, pb[:, :]
                )

            emit_phi(0)
            phi_queue = list(range(1, N // CK))

            # ------------- phases B and C (interleaved) -------------
            # wgT-raw matmuls are dribbled into the early B slots (2 per
            # slot); once done, psA closes and the z accumulator takes its
            # PSUM banks.
            state = {"z": None}
            wgt_queue = list(range(NMT))
            ets = [None] * NMT
            srecs = [None] * len(GROUP_SIZES)

            def emit_wgt(mt):
                ps = psA.tile([MTP, C], f32, tag="wgtps")
                nc.tensor.matmul(
                    ps,
                    lhsT=refa[:, mt * MTP : (mt + 1) * MTP],
                    rhs=wga[:, :],
                    start=True,
                    stop=True,
                )
                nc.vector.tensor_copy(wgt_raw[:, mt * C : (mt + 1) * C], ps)

            def emit_c(mt):
                g = group_of[mt]
                tl = mt - group_start[g]
                nc.vector.tensor_scalar_mul(
                    wgt_b16[:, mt * C : (mt + 1) * C],
                    wgt_raw[:, mt * C : (mt + 1) * C],
                    srecs[g][:, tl : tl + 1],
                )
                for j in range(NLOC // CK):
                    nc.tensor.matmul(
                        state["z"][:, j * CK : (j + 1) * CK],
                        lhsT=wgt_b16[:, mt * C : (mt + 1) * C],
                        rhs=ets[mt][:, j * CK : (j + 1) * CK],
                        start=(mt == 0),
                        stop=(mt == NMT - 1),
                    )

            # Estimated-time model for emission ordering: the PE executes
            # strictly in program order, so phase-C work for a tile must not
            # be emitted before its group's AllReduce has (by estimate)
            # landed, and at most one tile's C per slot to avoid starving
            # the fT matmuls that feed the (bottleneck) ACT exp stream.
            TILE_T = 2.7
            CC_LAT = 32.0
            CC_GAP = 10.0
            est = 0.0
            cc_land = [None] * len(GROUP_SIZES)
            c_ready = []

            for g, gs in enumerate(GROUP_SIZES):
                sA = spool.tile([MTP, gs], f32, tag=f"sA{g}")
                sB = spool.tile([MTP, gs], f32, tag=f"sB{g}")
                for tl in range(gs):
                    mt = group_start[g] + tl
                    et = epool.tile([MTP, NLOC], bf16, tag="et")
                    ets[mt] = et
                    for hh in range(2):
                        ft = ftp.tile([MTP, 2 * CK], f32, tag="ft")
                        for jj in range(2):
                            j = 2 * hh + jj
                            nc.tensor.matmul(
                                ft[:, jj * CK : (jj + 1) * CK],
                                lhsT=phi_t[:, mt * MTP : (mt + 1) * MTP],
                                rhs=theta_t[:, j * CK : (j + 1) * CK],
                                start=True,
                                stop=True,
                            )
                        acc = (sA if hh == 0 else sB)[:, tl : tl + 1]
                        nc.scalar.activation(
                            out=et[:, hh * 2 * CK : (hh + 1) * 2 * CK],
                            in_=ft,
                            func=AF.Exp,
                            accum_out=acc,
                        )
                    est += TILE_T
                    if phi_queue:
                        emit_phi(phi_queue.pop(0))
                    if wgt_queue:
                        emit_wgt(wgt_queue.pop(0))
                        if wgt_queue:
                            emit_wgt(wgt_queue.pop(0))
                        if not wgt_queue and not phi_queue:
                            psA_ctx.close()
                            zpp = ctx.enter_context(
                                tc.tile_pool(name="zpp", bufs=1, space="PSUM")
                            )
                            state["z"] = zpp.tile(
                                [C, NLOC], f32, tag="z", name="z_ps"
                            )
                    elif c_ready:
                        mt2 = c_ready[0]
                        land = cc_land[group_of[mt2]]
                        if mt2 == group_start[group_of[mt2]] and land is not None:
                            land += 2 * TILE_T
                        if land is not None and land <= est:
                            emit_c(c_ready.pop(0))
                # group complete: exchange softmax denominators
                stot = spool.tile([MTP, gs], f32, tag=f"stot{g}")
                nc.gpsimd.tensor_add(stot, sA, sB)
                cin = dpool.tile([MTP, gs], f32, tag=f"cin{g}")
                cout = dpool.tile([MTP, gs], f32, tag=f"cout{g}")
                nc.gpsimd.dma_start(out=cin, in_=stot)
                nc.gpsimd.collective_compute(
                    "AllReduce",
                    ALU.add,
                    replica_groups=REPLICA_GROUPS,
                    ins=[cin.opt()],
                    outs=[cout.opt()],
                )
                ssum = spool.tile([MTP, gs], f32, tag=f"ssum{g}")
                nc.sync.dma_start(out=ssum, in_=cout)
                srec = spool.tile([MTP, gs], f32, tag=f"srec{g}")
                nc.vector.reciprocal(out=srec, in_=ssum)
                srecs[g] = srec
                cc_land[g] = max(
                    est + CC_LAT,
                    (cc_land[g - 1] + CC_GAP) if g else 0.0,
                )
                c_ready.extend(range(group_start[g], group_start[g] + gs))

            while c_ready:
                emit_c(c_ready.pop(0))

            # ---------------- epilogue ----------------
            for j in range(NLOC // CK):
                e2 = outp.tile([C, CK], f32, tag="e2")
                # (z + w_b) + supp in one DVE op
                nc.vector.scalar_tensor_tensor(
                    out=e2,
                    in0=state["z"][:, j * CK : (j + 1) * CK],
                    scalar=wb[:, :],
                    in1=supp_t[:, j * CK : (j + 1) * CK],
                    op0=ALU.add,
                    op1=ALU.add,
                )
                eng = nc.sync if j % 2 == 0 else nc.scalar
                eng.dma_start(
                    out=out[:, j * CK : (j + 1) * CK], in_=e2
                )

    nc.compile()
    return nc


def _get_nc():
    if "nc" not in _cache:
        _cache["nc"] = _build()
    return _cache["nc"]


def kernel(
    supp_feature,
    ref_feature,
    theta_w,
    theta_b,
    phi_w,
    phi_b,
    g_w,
    g_b,
    w_w,
    w_b,
    _trace=False,
):
    import ml_dtypes

    # run_bass_kernel_spmd imports antenv.axon_hooks when tracing is
    # requested (e.g. via BASS_TRACE in the environment); this container's
    # antenv stub lacks that module, so provide a no-op fallback.
    try:
        import antenv.axon_hooks  # noqa: F401
    except ImportError:
        import sys
        import types

        import antenv

        _mod = types.ModuleType("antenv.axon_hooks")
        _mod._hook = None
        _mod.get_axon_ntff_profile_hook = lambda: _mod._hook
        _mod.set_axon_ntff_profile_hook = lambda h: setattr(_mod, "_hook", h)
        sys.modules["antenv.axon_hooks"] = _mod
        antenv.axon_hooks = _mod

    from concourse.bass_utils import run_bass_kernel_spmd

    bf = ml_dtypes.bfloat16
    supp_feature = np.asarray(supp_feature, dtype=np.float32)
    ref_feature = np.asarray(ref_feature, dtype=np.float32)
    theta_w = np.asarray(theta_w, dtype=np.float32)
    theta_b = np.asarray(theta_b, dtype=np.float32)
    phi_w = np.asarray(phi_w, dtype=np.float32)
    phi_b = np.asarray(phi_b, dtype=np.float32)
    g_w = np.asarray(g_w, dtype=np.float32)
    g_b = np.asarray(g_b, dtype=np.float32)
    w_w = np.asarray(w_w, dtype=np.float32)
    w_b = np.asarray(w_b, dtype=np.float32)

    nc = _get_nc()

    supp2 = supp_feature.reshape(B, C, N)
    ref2 = ref_feature.reshape(B, C, N)
    # Fold the output 1x1 conv into g (weight-only transform):
    #   w_w @ (g_w @ ref + g_b) = (w_w@g_w) @ ref + (w_w@g_b)
    Wg = (w_w @ g_w).astype(np.float32)
    wgb = (w_w @ g_b).astype(np.float32)
    wg_aug = np.ascontiguousarray(
        np.concatenate([Wg.T, wgb[None, :]], axis=0).astype(bf)
    )
    theta_wTh = np.ascontiguousarray(theta_w.T.astype(bf))
    phi_wTh = np.ascontiguousarray(phi_w.T.astype(bf))

    in_maps = []
    for core in range(NCORES):
        b, h = core // 2, core % 2
        ref_aug = np.ascontiguousarray(
            np.concatenate(
                [ref2[b], np.ones((1, N), np.float32)], axis=0
            ).astype(bf)
        )
        in_maps.append(
            {
                "supp": np.ascontiguousarray(
                    supp2[b, :, h * NLOC : (h + 1) * NLOC]
                ),
                "supp_b": np.ascontiguousarray(
                    supp2[b, :, h * NLOC : (h + 1) * NLOC].astype(bf)
                ),
                "ref_aug": ref_aug,
                "theta_wT": theta_wTh,
                "theta_bc": np.ascontiguousarray(theta_b.reshape(CI, 1)),
                "phi_wT": phi_wTh,
                "phi_bc": np.ascontiguousarray(phi_b.reshape(CI, 1)),
                "wg_aug": wg_aug,
                "w_bc": np.ascontiguousarray(w_b.reshape(C, 1)),
            }
        )

    res = run_bass_kernel_spmd(
        nc, in_maps, list(range(NCORES)), trace=_trace
    )
    if _trace:
        _cache["last_exec_time_ns"] = res.exec_time_ns
        _cache["last_results"] = res

    z = np.empty((B, C, N), dtype=np.float32)
    for core in range(NCORES):
        b, h = core // 2, core % 2
        z[b, :, h * NLOC : (h + 1) * NLOC] = res.results[core]["out"]
    return z.reshape(B, C, H, W)

